# revision 11
# baseline (speedup 1.0000x reference)
"""FCGRU + per-cultivar head (moe_routing) Trainium2 Bass kernel.

Math (see reference):
    x2 = input @ W1.T @ W2.T + ...       (two linear layers, no nonlinearity)
    gi = x2 @ W_ih.T + b_ih             -> folds into ONE matmul:
    gi = input @ Wgi.T + bgi,  Wgi = W_ih @ W2 @ W1 (K=512 instead of 512+1024)
    gh = h @ W_hh.T + b_hh
    r = sig(gi_r + gh_r); z = sig(gi_z + gh_z); n = tanh(gi_n + r * gh_n)
    h_new = n + z * (h - n)
    g2 = relu(relu(h_new) @ W3.T + b3)
    params[b] = g2[b] @ Wh[cult_b].T + bh[cult_b]

Distribution: data-parallel over batch on 8 cores. Host sorts the batch by
cultivar and deals samples round-robin to cores, so each (core, cultivar)
group is a contiguous run of <=CAP columns. The head is then 32 fixed-size
128-column windowed matmuls whose column start offsets are runtime values
(dynamic-offset DMA gather from a DRAM scratch of g2). Pad/garbage columns
inside a window get computed with the wrong head but are never read back:
the host gathers exactly one valid (cultivar, offset) slot per sample.

Layouts are feature-major ([feature_chunk*128+p] on partitions, batch on the
free axis) so no transposes are needed anywhere on device. Matmuls run in
bf16 with fp32 PSUM accumulation; all gate arithmetic is fp32.
"""

import numpy as np
import ml_dtypes

import concourse.bacc as bacc
import concourse.bass as bass
import concourse.mybir as mybir
import concourse.tile as tile
from concourse.bass_utils import run_bass_kernel_spmd

BF16 = mybir.dt.bfloat16
F32 = mybir.dt.float32
NP_BF16 = ml_dtypes.bfloat16

B, IN_DIM, HID, OUT, N_CULT = 16384, 512, 1024, 128, 32
DIM2 = HID // 2
N_CORES = 8

ts = bass.ts
ds = bass.ds


def build_nc(BC=2048, CTILE=512, CAP=128):
    """Build the per-core Bass kernel (SPMD: same program, 8 cores)."""
    NT = BC // CTILE
    assert BC % CTILE == 0

    nc = bacc.Bacc("TRN2")

    # ---- I/O ----
    xT = nc.dram_tensor("xT", [128, 4, BC], BF16, kind="ExternalInput")
    hTb = nc.dram_tensor("hTb", [128, 8, BC], BF16, kind="ExternalInput")
    hTf = nc.dram_tensor("hTf", [128, 8, BC], F32, kind="ExternalInput")
    WgiT = nc.dram_tensor("WgiT", [128, 4, 3 * HID], BF16, kind="ExternalInput")
    WhhT = nc.dram_tensor("WhhT", [128, 8, 3 * HID], BF16, kind="ExternalInput")
    W3T = nc.dram_tensor("W3T", [128, 8, DIM2], BF16, kind="ExternalInput")
    WhT = nc.dram_tensor("WhT", [N_CULT, 128, 4, OUT], BF16, kind="ExternalInput")
    bsum_rz = nc.dram_tensor("bsum_rz", [128, 16], F32, kind="ExternalInput")
    bgi_n = nc.dram_tensor("bgi_n", [128, 8], F32, kind="ExternalInput")
    bhh_n = nc.dram_tensor("bhh_n", [128, 8], F32, kind="ExternalInput")
    b3v = nc.dram_tensor("b3v", [128, 4], F32, kind="ExternalInput")
    bhv = nc.dram_tensor("bhv", [128, N_CULT], F32, kind="ExternalInput")
    starts = nc.dram_tensor("starts", [1, N_CULT], mybir.dt.int32, kind="ExternalInput")

    hnew_o = nc.dram_tensor("hnew_o", [128, 8, BC], F32, kind="ExternalOutput")
    params_o = nc.dram_tensor("params_o", [128, N_CULT * CAP], F32, kind="ExternalOutput")

    AF = mybir.ActivationFunctionType
    ALU = mybir.AluOpType

    with tile.TileContext(nc) as tc:
        with (
            tc.tile_pool(name="weights", bufs=1) as wpool,
            tc.tile_pool(name="psum", bufs=8, space="PSUM") as pspool,
            tc.tile_pool(name="acts", bufs=2) as apool,
            tc.tile_pool(name="small", bufs=3) as spool,
            tc.tile_pool(name="dram", bufs=1, space="DRAM") as dpool,
        ):
            # resident weights + biases
            Wgi_sb = wpool.tile([128, 4, 3 * HID], BF16)
            nc.sync.dma_start(Wgi_sb, WgiT[:, :, :])
            Whh_sb = wpool.tile([128, 8, 3 * HID], BF16)
            nc.sync.dma_start(Whh_sb, WhhT[:, :, :])
            W3_sb = wpool.tile([128, 8, DIM2], BF16)
            nc.sync.dma_start(W3_sb, W3T[:, :, :])
            bsrz_sb = wpool.tile([128, 16], F32)
            nc.sync.dma_start(bsrz_sb, bsum_rz[:, :])
            bgin_sb = wpool.tile([128, 8], F32)
            nc.sync.dma_start(bgin_sb, bgi_n[:, :])
            bhhn_sb = wpool.tile([128, 8], F32)
            nc.sync.dma_start(bhhn_sb, bhh_n[:, :])
            b3_sb = wpool.tile([128, 4], F32)
            nc.sync.dma_start(b3_sb, b3v[:, :])
            bhv_sb = wpool.tile([128, N_CULT], F32)
            nc.sync.dma_start(bhv_sb, bhv[:, :])
            starts_sb = wpool.tile([1, N_CULT], mybir.dt.int32)
            nc.sync.dma_start(starts_sb, starts[:, :])

            g2_dram = dpool.tile([128, 4, BC], BF16)

            for t in range(NT):
                cs = ts(t, CTILE)
                xt = apool.tile([128, 4, CTILE], BF16, tag="xt")
                nc.sync.dma_start(xt, xT[:, :, cs])
                htb = apool.tile([128, 8, CTILE], BF16, tag="htb")
                nc.sync.dma_start(htb, hTb[:, :, cs])
                g_rhs = apool.tile([128, 8, CTILE], BF16, tag="grhs")
                g2_sb = apool.tile([128, 4, CTILE], BF16, tag="g2")

                for j in range(8):
                    # r gate: feature chunk j of [0,1024)
                    ps_r = pspool.tile([128, CTILE], F32, tag="ps")
                    for k in range(4):
                        nc.tensor.matmul(ps_r, Wgi_sb[:, k, ts(j, 128)], xt[:, k],
                                         start=(k == 0), stop=False)
                    for k in range(8):
                        nc.tensor.matmul(ps_r, Whh_sb[:, k, ts(j, 128)], htb[:, k],
                                         start=False, stop=(k == 7))
                    r_j = spool.tile([128, CTILE], F32, tag="r")
                    nc.scalar.activation(r_j, ps_r, AF.Sigmoid, bias=bsrz_sb[:, j : j + 1])

                    # z gate: feature chunk 8+j
                    ps_z = pspool.tile([128, CTILE], F32, tag="ps")
                    for k in range(4):
                        nc.tensor.matmul(ps_z, Wgi_sb[:, k, ts(8 + j, 128)], xt[:, k],
                                         start=(k == 0), stop=False)
                    for k in range(8):
                        nc.tensor.matmul(ps_z, Whh_sb[:, k, ts(8 + j, 128)], htb[:, k],
                                         start=False, stop=(k == 7))
                    z_j = spool.tile([128, CTILE], F32, tag="z")
                    nc.scalar.activation(z_j, ps_z, AF.Sigmoid, bias=bsrz_sb[:, 8 + j : 9 + j])

                    # n gate: i_n and h_n kept separate (r multiplies h_n)
                    ps_in = pspool.tile([128, CTILE], F32, tag="ps")
                    for k in range(4):
                        nc.tensor.matmul(ps_in, Wgi_sb[:, k, ts(16 + j, 128)], xt[:, k],
                                         start=(k == 0), stop=(k == 3))
                    ps_hn = pspool.tile([128, CTILE], F32, tag="ps")
                    for k in range(8):
                        nc.tensor.matmul(ps_hn, Whh_sb[:, k, ts(16 + j, 128)], htb[:, k],
                                         start=(k == 0), stop=(k == 7))
                    t1 = spool.tile([128, CTILE], F32, tag="t1")
                    # t1 = (ps_hn + bhh_n) * r
                    nc.vector.scalar_tensor_tensor(t1, ps_hn, bhhn_sb[:, j : j + 1], r_j,
                                                   ALU.add, ALU.mult)
                    t2 = spool.tile([128, CTILE], F32, tag="t2")
                    # t2 = (ps_in + bgi_n) + t1
                    nc.vector.scalar_tensor_tensor(t2, ps_in, bgin_sb[:, j : j + 1], t1,
                                                   ALU.add, ALU.add)
                    n_j = spool.tile([128, CTILE], F32, tag="n")
                    nc.scalar.activation(n_j, t2, AF.Tanh)

                    # h_new = n + z * (h - n)
                    htf_j = spool.tile([128, CTILE], F32, tag="hf")
                    nc.sync.dma_start(htf_j, hTf[:, j, cs])
                    d_j = spool.tile([128, CTILE], F32, tag="d")
                    nc.vector.tensor_sub(d_j, htf_j, n_j)
                    nc.vector.tensor_mul(d_j, d_j, z_j)
                    hn_j = spool.tile([128, CTILE], F32, tag="hn")
                    nc.vector.tensor_add(hn_j, d_j, n_j)
                    nc.sync.dma_start(hnew_o[:, j, cs], hn_j)
                    # FC3 rhs: relu(h_new) cast to bf16
                    nc.scalar.activation(g_rhs[:, j], hn_j, AF.Relu)

                # FC3: g2 = relu(g_rhs @ W3.T + b3), out feature chunks m2
                for m2 in range(4):
                    ps = pspool.tile([128, CTILE], F32, tag="ps")
                    for k in range(8):
                        nc.tensor.matmul(ps, W3_sb[:, k, ts(m2, 128)], g_rhs[:, k],
                                         start=(k == 0), stop=(k == 7))
                    nc.scalar.activation(g2_sb[:, m2], ps, AF.Relu, bias=b3_sb[:, m2 : m2 + 1])
                nc.sync.dma_start(g2_dram[:, :, cs], g2_sb)

            # head: per cultivar, CAP-column window at runtime offset.
            # Dynamic-offset DMAs permanently consume a few registers each on
            # the issuing engine, so round-robin the 32 gathers across the
            # three engines that can issue them.
            gather_engines = [
                (nc.gpsimd, mybir.EngineType.Pool),
                (nc.sync, mybir.EngineType.SP),
                (nc.scalar, mybir.EngineType.Activation),
            ]
            for c in range(N_CULT):
                wc = spool.tile([128, 4, OUT], BF16, tag="whc")
                nc.sync.dma_start(wc, WhT[c])
                g_eng, g_eng_t = gather_engines[c % 3]
                start_c = nc.values_load(
                    starts_sb[0:1, c : c + 1],
                    engines=(g_eng_t,),
                    min_val=0,
                    max_val=BC - CAP,
                    skip_runtime_bounds_check=True,
                )
                bkt = spool.tile([128, 4, CAP], BF16, tag="bkt")
                g_eng.dma_start(bkt, g2_dram[:, :, ds(start_c, CAP)])
                ph = pspool.tile([128, CAP], F32, tag="ps")
                for k in range(4):
                    nc.tensor.matmul(ph, wc[:, k], bkt[:, k],
                                     start=(k == 0), stop=(k == 3))
                po = spool.tile([128, CAP], F32, tag="po")
                nc.scalar.activation(po, ph, AF.Identity, bias=bhv_sb[:, c : c + 1])
                nc.sync.dma_start(params_o[:, ts(c, CAP)], po)

    nc.compile()
    return nc


# ---------------------------------------------------------------------------
# host side
# ---------------------------------------------------------------------------

def _feature_major(a2d, kchunks):
    """[N, K*128] -> [128, kchunks, N] with feature f = k*128+p on (p, k)."""
    n = a2d.shape[0]
    return np.ascontiguousarray(a2d.reshape(n, kchunks, 128).transpose(2, 1, 0))


def prep_shared(W1, b1, W2, b2, W_ih, b_ih, W_hh, b_hh, W3, b3, Wh, bh):
    W1 = np.asarray(W1, np.float32)
    W2 = np.asarray(W2, np.float32)
    W_ih = np.asarray(W_ih, np.float32)
    W21 = W2 @ W1                       # [HID, IN_DIM]
    b21 = W2 @ np.asarray(b1, np.float32) + np.asarray(b2, np.float32)
    Wgi = W_ih @ W21                    # [3H, IN_DIM]
    bgi = W_ih @ b21 + np.asarray(b_ih, np.float32)
    b_hh = np.asarray(b_hh, np.float32)

    out = {}
    # lhsT layouts: [128, kchunks, M] with contraction feature k*128+p on (p,k)
    out["WgiT"] = np.ascontiguousarray(
        Wgi.reshape(3 * HID, 4, 128).transpose(2, 1, 0)).astype(NP_BF16)
    out["WhhT"] = np.ascontiguousarray(
        np.asarray(W_hh, np.float32).reshape(3 * HID, 8, 128).transpose(2, 1, 0)
    ).astype(NP_BF16)
    out["W3T"] = np.ascontiguousarray(
        np.asarray(W3, np.float32).reshape(DIM2, 8, 128).transpose(2, 1, 0)
    ).astype(NP_BF16)
    out["WhT"] = np.ascontiguousarray(
        np.asarray(Wh, np.float32).reshape(N_CULT, OUT, 4, 128).transpose(0, 3, 2, 1)
    ).astype(NP_BF16)
    out["bsum_rz"] = np.ascontiguousarray(
        (bgi[: 2 * HID] + b_hh[: 2 * HID]).reshape(16, 128).T)
    out["bgi_n"] = np.ascontiguousarray(bgi[2 * HID :].reshape(8, 128).T)
    out["bhh_n"] = np.ascontiguousarray(b_hh[2 * HID :].reshape(8, 128).T)
    out["b3v"] = np.ascontiguousarray(np.asarray(b3, np.float32).reshape(4, 128).T)
    out["bhv"] = np.ascontiguousarray(np.asarray(bh, np.float32).T)
    return out


_NC_CACHE = {}


def _get_nc(BC=2048, CTILE=512, CAP=128):
    key = (BC, CTILE, CAP)
    if key not in _NC_CACHE:
        _NC_CACHE[key] = build_nc(BC, CTILE, CAP)
    return _NC_CACHE[key]


def prepare(input, hn, cultivars, W1, b1, W2, b2, W_ih, W_hh, b_ih, b_hh, W3,
            b3, Wh, bh):
    BC = B // N_CORES
    CAP = 128
    input = np.asarray(input, np.float32)
    h0 = np.asarray(hn, np.float32)[0]
    cult = np.asarray(cultivars).reshape(-1).astype(np.int64)

    shared = prep_shared(W1, b1, W2, b2, W_ih, b_ih, W_hh, b_hh, W3, b3, Wh, bh)
    shared = {k: np.ascontiguousarray(v) for k, v in shared.items()}

    # sort by cultivar, deal round-robin to cores (keeps per-core sorted order
    # and balances every cultivar across cores)
    perm = np.argsort(cult, kind="stable")
    in_maps = []
    core_meta = []
    for j in range(N_CORES):
        idx = perm[j::N_CORES]
        cj = cult[idx]
        counts = np.bincount(cj, minlength=N_CULT)
        assert counts.max() <= CAP, (
            f"cultivar group of {counts.max()} samples on one core exceeds "
            f"head window capacity {CAP}")
        starts = np.zeros(N_CULT, np.int64)
        starts[1:] = np.cumsum(counts)[:-1]
        # clamp so every CAP-wide window is in-bounds; a clamped window still
        # covers its whole group (groups end at <= BC)
        starts = np.minimum(starts, BC - CAP)
        # slot (in the 32*CAP head output) of each sorted position
        slots = cj * CAP + (np.arange(BC) - starts[cj])

        xs = input[idx]                       # [BC, 512]
        hs = h0[idx]                          # [BC, 1024]
        m = dict(shared)
        m["xT"] = np.ascontiguousarray(
            xs.reshape(BC, 4, 128).transpose(2, 1, 0)).astype(NP_BF16)
        hT = np.ascontiguousarray(hs.reshape(BC, 8, 128).transpose(2, 1, 0))
        m["hTb"] = hT.astype(NP_BF16)
        m["hTf"] = hT
        m["starts"] = starts.astype(np.int32).reshape(1, N_CULT)
        in_maps.append(m)
        core_meta.append((idx, slots))

    nc = _get_nc(BC, 512, CAP)
    return nc, in_maps, core_meta


def run(trace=False, **inputs):
    nc, in_maps, core_meta = prepare(**inputs)
    res = run_bass_kernel_spmd(nc, in_maps, core_ids=list(range(N_CORES)),
                               trace=trace)

    BC = B // N_CORES
    params = np.empty((B, OUT), np.float32)
    hnew = np.empty((B, HID), np.float32)
    for j, (idx, slots) in enumerate(core_meta):
        r = res.results[j]
        # hnew_o [128, 8, BC]: feature f=c*128+p at [p, c, col]
        hnew[idx] = r["hnew_o"].transpose(2, 1, 0).reshape(BC, HID)
        params[idx] = r["params_o"][:, slots].T
    return (params, hnew[None]), res


def kernel(**inputs):
    outs, _ = run(**inputs)
    return outs
